# revision 12
# baseline (speedup 1.0000x reference)
"""AKConv + LKA fused Trainium2 kernel, batch-parallel across 8 NeuronCores.

Per-core (1 image, C=256, 64x64): offset conv (PE) -> bilinear index math (DVE,
pixel-major) -> PE-transpose fold into the gather's 16-partition-wrapped int16
index layout -> dma_gather deformable gather (pixel-major bf16) -> bilinear
combine (DVE) -> dma-transpose to channel-major -> ak conv (PE) -> BN batch
stats (+AllReduce) + SiLU (ACT) -> 5x5 dw + 7x7 dil3 dw as diag matmuls (PE)
-> 1x1 conv + gate -> final 1x1 conv + residual (PSUM-fused) -> out.

All DMAs keep contiguous per-partition runs (scattered APs blow up walrus's
IndirectLoad lowering); every layout permutation runs on PE transposes.
"""

import numpy as np
import ml_dtypes

import concourse.bass as bass
import concourse.bacc as bacc
import concourse.mybir as mybir
from concourse import tile
from concourse.bass_utils import run_bass_kernel_spmd
from concourse.library_config import mlp

BF16 = mybir.dt.bfloat16
F32 = mybir.dt.float32
I16 = mybir.dt.int16
I32 = mybir.dt.int32
AF = mybir.ActivationFunctionType
OP = mybir.AluOpType
AX = mybir.AxisListType

B, C, H, W = 8, 256, 64, 64
HW = H * W
NBLK = 32
EPS = 1e-5
P1 = 66   # pad-1 layout (offset conv input, residual source)
P2 = 68   # pad-2 layout (y_act, 5x5 dw input)
P9 = 82   # pad-9 layout (z1, 7x7 dil-3 dw input)


def _win(t, off, rstride, nrows=8, ncols=64):
    """[128, nrows, ncols] window AP into padded tile t at element offset off."""
    a = t[:] if not isinstance(t, bass.AP) else t
    return bass.AP(a.tensor, a.offset + off, [a.ap[0], [rstride, nrows], [1, ncols]])


def build_graph(n_cores: int):
    nc = bacc.Bacc(None, target_bir_lowering=False, num_swdge_queues=4)
    cores = list(range(n_cores))
    count = float(n_cores * HW)

    def par(name, shape, dt):
        return nc.declare_dram_parameter(name, shape, dt, isOutput=False)

    xpad = par("xpad", [2, 128, P1 * P1], BF16)
    xpm4 = par("xpm4", [128 * 128, 1024], BF16)
    basex = par("basex", [128, NBLK, 3], F32)
    basey = par("basey", [128, NBLK, 3], F32)
    woff = par("woff", [128, 108], BF16)         # [(k,t)*6] pre-laid
    pb = par("pb", [6, 1], F32)
    wak = par("wak", [128, 12 * 128], BF16)
    gamma = par("gamma", [128, 2], F32)
    beta = par("beta", [128, 2], F32)
    wd5 = par("wd5", [128, 32 * 128], BF16)   # PE taps 0..15 only
    wd7 = par("wd7", [128, 64 * 128], BF16)   # PE taps 0..31 only
    w5v = par("w5v", [128, 2, 25], F32)
    w7v = par("w7v", [128, 2, 17], F32)
    b5 = par("b5", [128, 2], F32)
    b7 = par("b7", [128, 2], F32)
    wlka1 = par("wlka1", [128, 4 * 128], BF16)
    blka1 = par("blka1", [128, 2], F32)
    wconv = par("wconv", [128, 4 * 128], BF16)
    bconv = par("bconv", [128, 2], F32)
    ident = par("ident", [128, 128], BF16)
    ident32 = par("ident32", [128, 128], F32)
    out = nc.declare_dram_parameter("out", [2, 128, HW], F32, isOutput=True)

    if n_cores > 1:
        stats_in = nc.dram_tensor("stats_in", [128, 4], F32)
        stats_out = nc.dram_tensor("stats_out", [128, 4], F32, addr_space="Shared")

    with tile.TileContext(nc) as tc:
        with (
            tc.tile_pool(name="const", bufs=1) as cp,
            tc.tile_pool(name="act", bufs=1) as ap_,
            tc.tile_pool(name="sm", bufs=1) as sp,
            tc.tile_pool(name="wk", bufs=2) as wp,
            tc.tile_pool(name="gt", bufs=3) as gp,
            tc.tile_pool(name="ga", bufs=2) as gacc,
            tc.tile_pool(name="ps", bufs=4, space="PSUM") as pp,
            tc.tile_pool(name="ps6", bufs=2, space="PSUM") as p6,
        ):
            nc.gpsimd.load_library(mlp)

            # ---------------- constant loads (all contiguous) ----------------
            def cload(name, shape, dt, src, tag=None):
                t = cp.tile(shape, dt, name=name, tag=tag or name)
                nc.sync.dma_start(out=t[:], in_=src)
                return t

            xpad_sb = [cload(f"xpad{k}", [128, P1 * P1], BF16, xpad[k])
                       for k in range(2)]
            woff_sb = cload("woffs", [128, 108], BF16, woff[:])
            wak_sb = cload("waks", [128, 12 * 128], BF16, wak[:])
            wlka1_sb = cload("wlka1s", [128, 4 * 128], BF16, wlka1[:])
            wconv_sb = cload("wconvs", [128, 4 * 128], BF16, wconv[:])
            ident_sb = cload("idents", [128, 128], BF16, ident[:])
            id32_sb = cload("id32s", [128, 128], F32, ident32[:])
            basex_sb = cload("basexs", [128, NBLK, 3], F32, basex[:])
            basey_sb = cload("baseys", [128, NBLK, 3], F32, basey[:])
            pb_sb = cload("pbs", [6, 1], F32, pb[:])
            w5v_sb = cload("w5vs", [128, 2, 25], F32, w5v[:])
            w7v_sb = cload("w7vs", [128, 2, 17], F32, w7v[:])
            small_params = {}
            for nm, h in (("gamma", gamma), ("beta", beta), ("b5", b5),
                          ("b7", b7), ("blka1", blka1), ("bconv", bconv)):
                small_params[nm] = cload(nm + "s", [128, 2], F32, h[:])

            # ---------------- stage 1: offset conv ----------------
            T_sb = sp.tile([128, NBLK, 6], F32, name="T_sb")
            for ct in range(8):
                ps = p6.tile([6, 512], F32, name="psoff", tag="psoff")
                i = 0
                for k in range(2):
                    for t in range(9):
                        dy, dx = t // 3, t % 3
                        nc.tensor.matmul(
                            out=ps[:],
                            lhsT=woff_sb[:, (k * 9 + t) * 6:(k * 9 + t) * 6 + 6],
                            rhs=_win(xpad_sb[k], (ct * 8 + dy) * P1 + dx, P1),
                            start=(i == 0), stop=(i == 17))
                        i += 1
                ob = wp.tile([6, 512], F32, name="ob", tag="ob")
                nc.scalar.activation(out=ob[:], in_=ps[:], func=AF.Identity,
                                     bias=pb_sb[:, 0:1])
                # transpose offsets to pixel-major [128, blk, 6]
                for j in range(4):
                    pt = p6.tile([128, 6], F32, name="ptr", tag="ptr")
                    nc.tensor.transpose(out=pt[:],
                                        in_=ob[:, j * 128:(j + 1) * 128],
                                        identity=id32_sb[:6, :6])
                    nc.vector.tensor_copy(out=T_sb[:, ct * 4 + j, :], in_=pt[:])

            # ---------------- stage 2: index + bilinear weight math ----------------
            def mk(tag):
                return sp.tile([128, NBLK, 3], F32, name=tag, tag=tag)

            def idx_side(base_sb, off_ap, lim, s):
                p = mk(f"p{s}")
                nc.vector.tensor_tensor(out=p[:], in0=base_sb[:], in1=off_ap, op=OP.add)
                t = mk(f"t{s}")
                nc.vector.tensor_scalar(out=t[:], in0=p[:], scalar1=1024.0,
                                        scalar2=None, op0=OP.add)
                ui = sp.tile([128, NBLK, 3], I32, name=f"ui{s}", tag=f"ui{s}")
                nc.vector.tensor_copy(out=ui[:], in_=t[:])
                uf = mk(f"uf{s}")
                nc.vector.tensor_copy(out=uf[:], in_=ui[:])
                nc.vector.tensor_tensor(out=t[:], in0=t[:], in1=uf[:], op=OP.subtract)
                nc.vector.tensor_scalar(out=t[:], in0=t[:], scalar1=0.0,
                                        scalar2=None, op0=OP.is_lt)
                qf = mk(f"qf{s}")
                nc.vector.scalar_tensor_tensor(out=qf[:], in0=uf[:], scalar=1024.0,
                                               in1=t[:], op0=OP.subtract,
                                               op1=OP.subtract)
                qlt = mk(f"qlt{s}")
                nc.vector.tensor_scalar(out=qlt[:], in0=qf[:], scalar1=0.0,
                                        scalar2=float(lim), op0=OP.max, op1=OP.min)
                qrb = mk(f"qrb{s}")
                nc.vector.tensor_scalar(out=qrb[:], in0=qf[:], scalar1=1.0,
                                        scalar2=0.0, op0=OP.add, op1=OP.max)
                nc.vector.tensor_scalar(out=qrb[:], in0=qrb[:], scalar1=float(lim),
                                        scalar2=None, op0=OP.min)
                pc = p
                nc.vector.tensor_scalar(out=pc[:], in0=p[:], scalar1=0.0,
                                        scalar2=float(lim), op0=OP.max, op1=OP.min)
                wlt = mk(f"wlt{s}")
                nc.vector.scalar_tensor_tensor(out=wlt[:], in0=qlt[:], scalar=1.0,
                                               in1=pc[:], op0=OP.add, op1=OP.subtract)
                wrb = mk(f"wrb{s}")
                nc.vector.scalar_tensor_tensor(out=wrb[:], in0=pc[:], scalar=1.0,
                                               in1=qrb[:], op0=OP.add, op1=OP.subtract)
                return qlt, qrb, wlt, wrb

            qlx, qrx, wxl, wxr = idx_side(basex_sb, T_sb[:, :, 0:3], H - 1, "x")
            qly, qry, wyl, wyr = idx_side(basey_sb, T_sb[:, :, 3:6], W - 1, "y")

            gw = []
            for gi, (wx, wy) in enumerate(((wxl, wyl), (wxr, wyr),
                                           (wxl, wyr), (wxr, wyl))):
                g = mk(f"g{gi}")
                nc.vector.tensor_tensor(out=g[:], in0=wx[:], in1=wy[:], op=OP.mult)
                gw.append(g)

            # single patch index per (tap, pixel): idx = (qlx+qrx)*128 + (qly+qry)
            wi16 = sp.tile([16, 3, 256], I16, name="wi16")
            rx = mk("rx")
            nc.vector.tensor_tensor(out=rx[:], in0=qlx[:], in1=qrx[:], op=OP.add)
            ry = mk("ry")
            nc.vector.tensor_tensor(out=ry[:], in0=qly[:], in1=qry[:], op=OP.add)
            uf_t = sp.tile([128, 96], F32, name="uft", tag="uft")
            for n in range(3):
                nc.vector.scalar_tensor_tensor(
                    out=uf_t[:, n * 32:(n + 1) * 32],
                    in0=rx[:, :, n], scalar=128.0, in1=ry[:, :, n],
                    op0=OP.mult, op1=OP.add)
            # T1: V[(n,blk), pp] = Uf[pp, (n,blk)]
            pv = p6.tile([96, 128], F32, name="ptr", tag="ptr")
            nc.tensor.transpose(out=pv[:], in_=uf_t[:], identity=id32_sb[:])
            v_t = sp.tile([96, 128], F32, name="vt", tag="vt")
            nc.vector.tensor_copy(out=v_t[:], in_=pv[:])
            # T2 per pphi: W[q, (n,blk)] = V[(n,blk), 16*pphi + q]
            for ph in range(8):
                pw = p6.tile([16, 96], F32, name="ptr", tag="ptr")
                nc.tensor.transpose(out=pw[:],
                                    in_=v_t[:, 16 * ph:16 * ph + 16],
                                    identity=id32_sb[:96, :96])
                for n in range(3):
                    dst = wi16[:, n, :]
                    dst = bass.AP(dst.tensor, dst.offset + ph,
                                  [dst.ap[0], [8, 32]])
                    nc.vector.tensor_copy(out=dst,
                                          in_=pw[:, n * 32:(n + 1) * 32])
            idxw = sp.tile([128, 3, 256], I16, name="idxw")
            for g8 in range(8):
                nc.sync.dma_start(out=idxw[16 * g8:16 * g8 + 16, :, :],
                                  in_=wi16[:, :, :])

            # ---------------- stage 3/4/5: pipelined gather + combine +
            # transpose + ak conv + BN stats, q-major so gathers overlap
            # combine and the ak-conv matmuls overlap the next q's gathers.
            xoff = [ap_.tile([128, 2, HW], BF16, name=f"xoff{n}", tag=f"xoff{n}")
                    for n in range(3)]
            ypre = ap_.tile([128, 2, HW], BF16, name="ypre", tag="ypre")
            sump = sp.tile([128, 2, 8], F32, name="sump")
            sqp = sp.tile([128, 2, 8], F32, name="sqp")
            for q in range(4):
                for n in range(3):
                    j = q * 3 + n
                    g_ = gp.tile([128, 8, 1024], BF16, name="G", tag=f"G{j % 3}",
                                 bufs=1)
                    nc.gpsimd.dma_gather(
                        g_[:], xpm4[:],
                        idxw[:, n, 64 * q:64 * q + 64],
                        1024, 1024, 1024, queue_num=j % 4)
                    acc = gacc.tile([128, 8, C], BF16, name="acc", tag="acc")
                    for b_ in range(8):
                        bg = q * 8 + b_
                        nc.vector.tensor_scalar(
                            out=acc[:, b_, :], in0=g_[:, b_, 0:256],
                            scalar1=gw[0][:, bg, n:n + 1], scalar2=None, op0=OP.mult)
                        for ci in range(1, 4):
                            nc.vector.scalar_tensor_tensor(
                                out=acc[:, b_, :],
                                in0=g_[:, b_, ci * 256:(ci + 1) * 256],
                                scalar=gw[ci][:, bg, n:n + 1],
                                in1=acc[:, b_, :], op0=OP.mult, op1=OP.add)
                    for b_ in range(8):
                        bg = q * 8 + b_
                        nc.sync.dma_start_transpose(
                            out=xoff[n][:, :, bg * 128:bg * 128 + 128],
                            in_=acc[:, b_, :])
                for ct in (2 * q, 2 * q + 1):
                    for m in range(2):
                        ps = pp.tile([128, 512], F32, name="mm", tag="mm")
                        i = 0
                        for n in range(3):
                            for k in range(2):
                                nc.tensor.matmul(
                                    out=ps[:],
                                    lhsT=wak_sb[:, (n * 4 + k * 2 + m) * 128:
                                                (n * 4 + k * 2 + m) * 128 + 128],
                                    rhs=xoff[n][:, k, ct * 512:(ct + 1) * 512],
                                    start=(i == 0), stop=(i == 5))
                                i += 1
                        ysl = ypre[:, m, ct * 512:(ct + 1) * 512]
                        nc.scalar.activation(out=ysl, in_=ps[:], func=AF.Copy,
                                             accum_out=sump[:, m, ct:ct + 1])
                        sq_ps = p6.tile([128, 512], F32, name="sqs", tag="psoff")
                        nc.scalar.activation(out=sq_ps[:], in_=ps[:],
                                             func=AF.Square,
                                             accum_out=sqp[:, m, ct:ct + 1])
            stats_sb = sp.tile([128, 4], F32, name="stats_sb")
            for m in range(2):
                nc.vector.tensor_reduce(out=stats_sb[:, 2 * m:2 * m + 1],
                                        in_=sump[:, m, :], axis=AX.X, op=OP.add)
                nc.vector.tensor_reduce(out=stats_sb[:, 2 * m + 1:2 * m + 2],
                                        in_=sqp[:, m, :], axis=AX.X, op=OP.add)
            if n_cores > 1:
                nc.gpsimd.dma_start(out=stats_in[:], in_=stats_sb[:])
                nc.gpsimd.collective_compute(
                    "AllReduce", OP.add, replica_groups=[cores],
                    ins=[stats_in[:]], outs=[stats_out[:]])

            # stats-independent work issued here so it overlaps the AllReduce:
            # dw weight load, yact/z1 zero-fills, eps constant.
            yact = [ap_.tile([128, P2 * P2], BF16, name=f"yact{m}", tag=f"yact{m}")
                    for m in range(2)]
            for m in range(2):
                nc.vector.memset(yact[m][:], 0.0)
            wd5_sb = ap_.tile([128, 32 * 128], BF16, name="wdw", tag="ypre2")
            nc.sync.dma_start(out=wd5_sb[:], in_=wd5[:])
            z1 = [ap_.tile([128, P9 * P9], BF16, name=f"z1{m}", tag=f"xoff{m}")
                  for m in range(2)]
            for m in range(2):
                nc.vector.memset(z1[m][:], 0.0)
            epsc = sp.tile([128, 1], F32, name="epsc")
            nc.vector.memset(epsc[:], EPS)

            if n_cores > 1:
                stats_all = sp.tile([128, 4], F32, name="stats_all")
                nc.gpsimd.dma_start(out=stats_all[:], in_=stats_out[:])
            else:
                stats_all = stats_sb

            mean = sp.tile([128, 2], F32, name="mean")
            ex2 = sp.tile([128, 2], F32, name="ex2")
            sa = stats_all[:]
            nc.vector.tensor_scalar(
                out=mean[:], in0=bass.AP(sa.tensor, sa.offset, [sa.ap[0], [2, 2]]),
                scalar1=1.0 / count, scalar2=None, op0=OP.mult)
            nc.vector.tensor_scalar(
                out=ex2[:], in0=bass.AP(sa.tensor, sa.offset + 1, [sa.ap[0], [2, 2]]),
                scalar1=1.0 / count, scalar2=None, op0=OP.mult)
            negv = sp.tile([128, 2], F32, name="negv")
            for m in range(2):
                nc.vector.scalar_tensor_tensor(
                    out=negv[:, m:m + 1], in0=mean[:, m:m + 1],
                    scalar=mean[:, m:m + 1], in1=ex2[:, m:m + 1],
                    op0=OP.mult, op1=OP.subtract)
            std = sp.tile([128, 2], F32, name="std")
            nc.scalar.activation(out=std[:], in_=negv[:], func=AF.Sqrt,
                                 bias=epsc[:, 0:1], scale=-1.0)
            inv = sp.tile([128, 2], F32, name="inv")
            nc.vector.reciprocal(out=inv[:], in_=std[:])
            scale = sp.tile([128, 2], F32, name="scale")
            nc.vector.tensor_tensor(out=scale[:], in0=small_params["gamma"][:],
                                    in1=inv[:], op=OP.mult)
            nsc = sp.tile([128, 2], F32, name="nsc")
            nc.vector.tensor_scalar(out=nsc[:], in0=scale[:], scalar1=-1.0,
                                    scalar2=None, op0=OP.mult)
            shift = sp.tile([128, 2], F32, name="shift")
            for m in range(2):
                nc.vector.scalar_tensor_tensor(
                    out=shift[:, m:m + 1], in0=mean[:, m:m + 1],
                    scalar=nsc[:, m:m + 1], in1=small_params["beta"][:, m:m + 1],
                    op0=OP.mult, op1=OP.add)

            for m in range(2):
                for ct in range(8):
                    ysl = ypre[:, m, ct * 512:(ct + 1) * 512]
                    sg = wp.tile([128, 512], BF16, name="sgt", tag="sgt")
                    nc.scalar.activation(out=sg[:], in_=ysl, func=AF.Sigmoid,
                                         bias=shift[:, m:m + 1],
                                         scale=scale[:, m:m + 1])
                    yb = wp.tile([128, 512], BF16, name="ybn", tag="ybn")
                    nc.scalar.activation(out=yb[:], in_=ysl, func=AF.Identity,
                                         bias=shift[:, m:m + 1],
                                         scale=scale[:, m:m + 1])
                    nc.vector.tensor_tensor(
                        out=_win(yact[m], (ct * 8 + 2) * P2 + 2, P2),
                        in0=bass.AP(yb[:].tensor, yb[:].offset,
                                    [yb[:].ap[0], [64, 8], [1, 64]]),
                        in1=bass.AP(sg[:].tensor, sg[:].offset,
                                    [sg[:].ap[0], [64, 8], [1, 64]]),
                        op=OP.mult)

            # ---------------- stage 6: depthwise 5x5 (diag matmuls) ----------------
            D5 = 16  # taps [0, D5) on PE, [D5, 25) on DVE
            for m in range(2):
                for ct in range(8):
                    ps = pp.tile([128, 512], F32, name="mm", tag="mm")
                    for t in range(D5):
                        dy, dx = t // 5, t % 5
                        nc.tensor.matmul(
                            out=ps[:],
                            lhsT=wd5_sb[:, (t * 2 + m) * 128:(t * 2 + m) * 128 + 128],
                            rhs=_win(yact[m], (ct * 8 + dy) * P2 + dx, P2),
                            start=(t == 0), stop=(t == D5 - 1))
                    dacc = wp.tile([128, 512], BF16, name="dacc", tag="dacc")
                    for t in range(D5, 25):
                        dy, dx = t // 5, t % 5
                        w_ = _win(yact[m], (ct * 8 + dy) * P2 + dx, P2)
                        sc = w5v_sb[:, m, t:t + 1]
                        if t == D5:
                            nc.vector.tensor_scalar(out=dacc[:], in0=w_, scalar1=sc,
                                                    scalar2=None, op0=OP.mult)
                        else:
                            nc.vector.scalar_tensor_tensor(
                                out=dacc[:], in0=w_, scalar=sc, in1=dacc[:],
                                op0=OP.mult, op1=OP.add)
                    nc.vector.scalar_tensor_tensor(
                        out=_win(z1[m], (ct * 8 + 9) * P9 + 9, P9),
                        in0=ps[:], scalar=small_params["b5"][:, m:m + 1],
                        in1=dacc[:], op0=OP.add, op1=OP.add)

            # ---------------- stage 7: depthwise 7x7 dilation 3 ----------------
            wd7_sb = ap_.tile([128, 64 * 128], BF16, name="wdw2", tag="ypre2")
            nc.sync.dma_start(out=wd7_sb[:], in_=wd7[:])
            z2 = ap_.tile([128, 2, HW], BF16, name="z2", tag="xoff2")
            D7 = 32  # taps [0, D7) on PE, [D7, 49) on DVE
            for m in range(2):
                for ct in range(8):
                    ps = pp.tile([128, 512], F32, name="mm", tag="mm")
                    for t in range(D7):
                        dy, dx = (t // 7) * 3, (t % 7) * 3
                        nc.tensor.matmul(
                            out=ps[:],
                            lhsT=wd7_sb[:, (t * 2 + m) * 128:(t * 2 + m) * 128 + 128],
                            rhs=_win(z1[m], (ct * 8 + dy) * P9 + dx, P9),
                            start=(t == 0), stop=(t == D7 - 1))
                    dacc = wp.tile([128, 512], BF16, name="dacc", tag="dacc")
                    for t in range(D7, 49):
                        dy, dx = (t // 7) * 3, (t % 7) * 3
                        w_ = _win(z1[m], (ct * 8 + dy) * P9 + dx, P9)
                        sc = w7v_sb[:, m, t - D7:t - D7 + 1]
                        if t == D7:
                            nc.vector.tensor_scalar(out=dacc[:], in0=w_, scalar1=sc,
                                                    scalar2=None, op0=OP.mult)
                        else:
                            nc.vector.scalar_tensor_tensor(
                                out=dacc[:], in0=w_, scalar=sc, in1=dacc[:],
                                op0=OP.mult, op1=OP.add)
                    nc.vector.scalar_tensor_tensor(
                        out=z2[:, m, ct * 512:(ct + 1) * 512],
                        in0=ps[:], scalar=small_params["b7"][:, m:m + 1],
                        in1=dacc[:], op0=OP.add, op1=OP.add)

            # ---------------- stage 8: 1x1 conv + gate ----------------
            gated = ap_.tile([128, 2, HW], BF16, name="gated", tag="ypre2")
            for m in range(2):
                for ct in range(8):
                    ps = pp.tile([128, 512], F32, name="mm", tag="mm")
                    for k in range(2):
                        nc.tensor.matmul(
                            out=ps[:],
                            lhsT=wlka1_sb[:, (k * 2 + m) * 128:(k * 2 + m) * 128 + 128],
                            rhs=z2[:, k, ct * 512:(ct + 1) * 512],
                            start=(k == 0), stop=(k == 1))
                    nc.vector.scalar_tensor_tensor(
                        out=gated[:, m, ct * 512:(ct + 1) * 512], in0=ps[:],
                        scalar=small_params["blka1"][:, m:m + 1],
                        in1=_win(yact[m], (ct * 8 + 2) * P2 + 2, P2),
                        op0=OP.add, op1=OP.mult)

            # ---------------- stage 9: final 1x1 conv + residual ----------------
            for m in range(2):
                for ct in range(8):
                    ps = pp.tile([128, 512], F32, name="mm", tag="mm")
                    for k in range(2):
                        nc.tensor.matmul(
                            out=ps[:],
                            lhsT=wconv_sb[:, (k * 2 + m) * 128:(k * 2 + m) * 128 + 128],
                            rhs=gated[:, k, ct * 512:(ct + 1) * 512],
                            start=(k == 0), stop=False)
                    nc.tensor.matmul(
                        out=ps[:], lhsT=ident_sb[:],
                        rhs=_win(xpad_sb[m], (ct * 8 + 1) * P1 + 1, P1),
                        start=False, stop=True)
                    osb = wp.tile([128, 512], F32, name="osb", tag="osb")
                    nc.scalar.activation(out=osb[:], in_=ps[:], func=AF.Identity,
                                         bias=small_params["bconv"][:, m:m + 1])
                    nc.sync.dma_start(out=out[m][:, ct * 512:(ct + 1) * 512],
                                      in_=osb[:])

    nc.compile()
    return nc


_CACHE = {}


def _pack(inputs, n_cores):
    bf = ml_dtypes.bfloat16
    x = np.asarray(inputs["x"], np.float32)
    p_w = np.asarray(inputs["p_w"], np.float32)
    ak_w = np.asarray(inputs["ak_w"], np.float32)

    woff = np.zeros((128, 18, 6), np.float32)
    for k in range(2):
        for t in range(9):
            woff[:, k * 9 + t, :] = p_w[:, k * 128:(k + 1) * 128, t // 3, t % 3].T
    wak = np.zeros((128, 12, 128), np.float32)
    for n in range(3):
        for k in range(2):
            for m in range(2):
                wak[:, n * 4 + k * 2 + m, :] = ak_w[m * 128:(m + 1) * 128,
                                                    k * 128:(k + 1) * 128, n, 0].T

    def diag_flat(w2d, taps):  # -> [128, taps*2, 128]
        o = np.zeros((128, taps * 2, 128), np.float32)
        idx = np.arange(128)
        for t in range(taps):
            for m in range(2):
                o[idx, t * 2 + m, idx] = w2d[m * 128:(m + 1) * 128, t]
        return o.reshape(128, -1).astype(bf)

    def chunk2(v):
        return np.asarray(v, np.float32).reshape(2, 128).T.copy()

    def onebyone(w):
        w = np.asarray(w, np.float32).reshape(C, C)
        o = np.zeros((128, 4, 128), np.float32)
        for k in range(2):
            for m in range(2):
                o[:, k * 2 + m, :] = w[m * 128:(m + 1) * 128,
                                       k * 128:(k + 1) * 128].T
        return o.reshape(128, -1).astype(bf)

    pp_, blk = np.meshgrid(np.arange(128), np.arange(NBLK), indexing="ij")
    pix = blk * 128 + pp_
    basex = ((pix // 64)[:, :, None] + np.array([0., 0., 1.])[None, None, :])
    basey = ((pix % 64)[:, :, None] + np.array([0., 1., 0.])[None, None, :])

    shared = dict(
        basex=basex.astype(np.float32), basey=basey.astype(np.float32),
        woff=woff.reshape(128, 108).astype(bf),
        pb=np.asarray(inputs["p_b"], np.float32).reshape(6, 1),
        wak=wak.reshape(128, -1).astype(bf),
        gamma=chunk2(inputs["ak_gamma"]), beta=chunk2(inputs["ak_beta"]),
        wd5=diag_flat(np.asarray(inputs["lka0_w"], np.float32).reshape(C, 25), 16),
        wd7=diag_flat(np.asarray(inputs["lkas_w"], np.float32).reshape(C, 49), 32),
        w5v=np.asarray(inputs["lka0_w"], np.float32).reshape(C, 25)
            .reshape(2, 128, 25).transpose(1, 0, 2).copy(),
        w7v=np.asarray(inputs["lkas_w"], np.float32).reshape(C, 49)[:, 32:]
            .reshape(2, 128, 17).transpose(1, 0, 2).copy(),
        b5=chunk2(inputs["lka0_b"]), b7=chunk2(inputs["lkas_b"]),
        wlka1=onebyone(inputs["lka1_w"]), blka1=chunk2(inputs["lka1_b"]),
        wconv=onebyone(inputs["conv_w"]), bconv=chunk2(inputs["conv_b"]),
        ident=np.eye(128, dtype=np.float32).astype(bf),
        ident32=np.eye(128, dtype=np.float32),
    )

    in_maps = []
    for i in range(n_cores):
        xi = x[i].reshape(C, H, W)
        xp1 = np.zeros((C, P1, P1), np.float32)
        xp1[:, 1:65, 1:65] = xi
        m = dict(shared)
        m["xpad"] = xp1.reshape(2, 128, P1 * P1).astype(bf)
        r = np.arange(128)
        f, c = r // 2, np.minimum((r + 1) // 2, 63)
        xf = xi[:, f, :]          # (C, 128, 64)
        xc = xi[:, c, :]
        p4 = np.empty((128, 128, 4, C), np.float32)
        p4[:, :, 0, :] = xf[:, :, f].transpose(1, 2, 0)   # lt (fx, fy)
        p4[:, :, 1, :] = xc[:, :, c].transpose(1, 2, 0)   # rb (cx, cy)
        p4[:, :, 2, :] = xf[:, :, c].transpose(1, 2, 0)   # lb (fx, cy)
        p4[:, :, 3, :] = xc[:, :, f].transpose(1, 2, 0)   # rt (cx, fy)
        m["xpm4"] = p4.reshape(128 * 128, 1024).astype(bf)
        in_maps.append(m)
    return in_maps


def kernel(**inputs) -> np.ndarray:
    n_cores = 8
    if n_cores not in _CACHE:
        _CACHE[n_cores] = build_graph(n_cores)
    nc = _CACHE[n_cores]
    in_maps = _pack(inputs, n_cores)
    res = run_bass_kernel_spmd(nc, in_maps, list(range(n_cores)))
    outs = [np.asarray(res.results[i]["out"], np.float32).reshape(C, H, W)
            for i in range(n_cores)]
    return np.stack(outs).astype(np.float32)

